# revision 7
# baseline (speedup 1.0000x reference)
"""Multihead attention (B=4, S=2048, D=1024, H=16) on 8 Trainium2 NeuronCores.

V3: head-PAIR blocks with PE tile-position concurrency (HW-validated):
  - scores: K=64 matmuls for heads (2p, 2p+1) at row offsets 0/64 issued
    adjacently -> run concurrently on PE row-tile halves (1.93x measured).
  - AV: M=64 matmuls for the head pair col-tiled at out partitions 0/64
    (1.93x measured); softmax denominators via a 4-way col-tiled M=1
    ones-matmul quad at out partitions 0/32/64/96 (4x measured).
  - PE work/core: 131K (scores) + 131K (AV) + 65K (denom) + 197K (proj)
    + 65K (oproj) = 590K cycles vs 786K in v2.
  - exp: ScalarE activation for most tiles; a fraction offloaded to a
    custom-DVE 4-pass exp (range-reduce, 2^k-bits via scaled-magic +
    int32 output cast, squared-quadratic poly, stock multiply) to pull
    ScalarE (the v2 bottleneck at ~290us) below the PE roofline.
    Avoids HW-crashing custom-DVE constructs (Src1 [P,1] operands and
    triple stream reads -- found by bisection; CoreSim accepts them).
  - normalize: denom rows DMA'd psum->sbuf@p0, reciprocal_approx_fast
    (5x faster than nc.vector.reciprocal, which cost 107us/core in v2),
    gpsimd partition_broadcast, DVE multiplies.
  - Q/K proj bias adds + V bias add moved DVE->GpSimd (idle engine).

Sharding: data-parallel over batch (4) x tensor-parallel over heads (2
groups of 8). Core c: batch c//2, head-group c%2. Out-proj partials
summed on host.

PSUM (8 banks): sA/sB [128,1024]f32 (4), av j0/j1 [128,512]f32 (2),
denom [128,512]f32 (1), proj/oproj [128,512]f32 (1).

HW gotchas: reciprocal output / partition_broadcast source must sit at
SBUF base partition 0; tile_position col 96 must be passed explicitly.
"""

import sys

if "/opt/trn_rl_repo" not in sys.path:
    sys.path.insert(0, "/opt/trn_rl_repo")

import math

import numpy as np
import ml_dtypes

P = 128
S = 2048
DIN = 1024
DG = 512
HD = 64
NH_LOCAL = 8
N_CORES = 8

LOG2E_8 = float(np.log2(np.e) / 8.0)
MAGIC = 12582912.0                      # 1.5 * 2^23
LN2_8 = float(8.0 * np.log(2.0))
C0B = float(LOG2E_8 * 8388608.0)        # (log2e/8) * 2^23
C1B = float(1.5 * 2 ** 46)
C2B = float(12582785.0 * 8388608.0)     # (MAGIC - 127) * 2^23

OFFLOAD_MOD = 5     # every MOD-th (block,kc) slot sends one exp to DVE; 0=off
STREAM = True

_CACHE: dict = {}


def register_exp_ops():
    """Register the custom-DVE exp ops (idempotent, runtime registration)."""
    import concourse.dve_ops as DO
    from concourse.dve_spec import (Spec, Src0, C0, C1, C2, One, sq,
                                    lower as dlower, _has_src1)
    from concourse.dve_uop import DveOpSpec

    if "EXPR_ANT" in DO._SUB_OPCODE_FOR_NAME:
        ops = {op.name: op for op in DO.OPS}
        return ops["EXPR_ANT"], ops["EXPK_ANT"], ops["EXPQ_ANT"]

    def reg(name, spec):
        shas = {}
        for ver in ("v3", "v4"):
            try:
                uops = dlower(spec, ver=ver)
                shas[ver] = DveOpSpec(name=name, opcode=0, uops=uops,
                                      rd1_en=_has_src1(spec)).sha(ver)
            except Exception:
                pass
        op = DO.DveOp(name, spec, False, uops_sha=shas)
        DO.OPS.append(op)
        DO._SUB_OPCODE_FOR_NAME[name] = DO._CUSTOM_DVE_ROW_BASE + len(DO.OPS) - 1
        DO.CUSTOM_DVE_SPECS[name] = spec
        return op

    _k = (Src0 * C0 + C1) - C1
    RP = reg("EXPR_ANT", Spec(
        body=Src0 - _k * C2,
        reference=lambda in0, in1, s0, s1, imm2:
            in0 - ((in0 * s0 + s1) - s1) * imm2))
    PK = reg("EXPK_ANT", Spec(
        body=(Src0 * C0 + C1) - C2,
        reference=lambda in0, in1, s0, s1, imm2: (in0 * s0 + s1) - imm2))
    _v = Src0 * C0
    _q = (_v * C1 + One) * _v + One
    QS = reg("EXPQ_ANT", Spec(
        body=sq(_q),
        reference=lambda in0, in1, s0, s1, imm2:
            ((in0 * s0 * s1 + 1.0) * (in0 * s0) + 1.0) ** 2))
    return RP, PK, QS


def build_bass(repeat: int = 1):
    from concourse import bacc, tile, mybir

    f32 = mybir.dt.float32
    bf16 = mybir.dt.bfloat16

    nc = bacc.Bacc("TRN2", target_bir_lowering=False, debug=False,
                   num_devices=N_CORES)

    xqT = nc.dram_tensor("xqT", [DIN, S], bf16, kind="ExternalInput")
    xkT = nc.dram_tensor("xkT", [DIN, S], bf16, kind="ExternalInput")
    xvT = nc.dram_tensor("xvT", [DIN, S], bf16, kind="ExternalInput")
    wqT = nc.dram_tensor("wqT", [DIN, DG], bf16, kind="ExternalInput")
    wkT = nc.dram_tensor("wkT", [DIN, DG], bf16, kind="ExternalInput")
    wvT = nc.dram_tensor("wvT", [DIN, DG], bf16, kind="ExternalInput")
    woT = nc.dram_tensor("woT", [DG, DIN], bf16, kind="ExternalInput")
    bqd = nc.dram_tensor("bq", [P, 4], f32, kind="ExternalInput")
    bkd = nc.dram_tensor("bk", [P, 4], f32, kind="ExternalInput")
    bvd = nc.dram_tensor("bv", [1, DG], f32, kind="ExternalInput")
    outp = nc.dram_tensor("outp", [S, DIN], bf16, kind="ExternalOutput")

    with tile.TileContext(nc) as tc:
        for _ in range(repeat):
            _emit(nc, tc, xqT, xkT, xvT, wqT, wkT, wvT, woT, bqd, bkd, bvd,
                  outp)
    nc.compile()
    return nc


def _emit(nc, tc, xqT, xkT, xvT, wqT, wkT, wvT, woT, bqd, bkd, bvd, outp):
    from concourse import mybir

    f32 = mybir.dt.float32
    bf16 = mybir.dt.bfloat16
    i32 = mybir.dt.int32
    Exp = mybir.ActivationFunctionType.Exp
    Copy = mybir.ActivationFunctionType.Copy
    mult = mybir.AluOpType.mult
    add_op = mybir.AluOpType.add

    RP, PK, QS = register_exp_ops()

    with (
        tc.tile_pool(name="consts", bufs=1) as consts,
        tc.tile_pool(name="xin", bufs=3) as xin,
        tc.tile_pool(name="qkv", bufs=1) as qkvp,
        tc.tile_pool(name="attn", bufs=5) as attnp,
        tc.tile_pool(name="small", bufs=1) as smallp,
        tc.tile_pool(name="dvp", bufs=1) as dvp,
        tc.tile_pool(name="osb", bufs=1) as osbp,
        tc.tile_pool(name="psA", bufs=1, space="PSUM") as psA,
        tc.tile_pool(name="psB", bufs=1, space="PSUM") as psB,
        tc.tile_pool(name="pav", bufs=1, space="PSUM") as pav,
        tc.tile_pool(name="pdn", bufs=1, space="PSUM") as pdn,
        tc.tile_pool(name="pq", bufs=1, space="PSUM") as pq,
    ):
        QT = qkvp.tile([P, 4, S], bf16, tag="QT")
        KT = qkvp.tile([P, 4, S], bf16, tag="KT")
        vt = qkvp.tile([P, 16, DG], bf16, tag="vt")
        OT = qkvp.tile([P, 4, S], bf16, tag="OT")
        ones = consts.tile([P, 1], bf16, tag="ones")
        nc.vector.memset(ones[:], 1.0)

        def alloc_x(nm):
            return xin.tile([P, 8, S], bf16, tag="x", name=nm)

        def load_x_st(xdram, xt_sb, st):
            xt = xdram.ap().rearrange("(c p) m -> p c m", p=P)
            sl = slice(st * 512, (st + 1) * 512)
            nc.sync.dma_start(xt_sb[:, 0:4, sl], xt[:, 0:4, sl])
            nc.gpsimd.dma_start(xt_sb[:, 4:8, sl], xt[:, 4:8, sl])

        def w_bias(wdram, bdram, wtag, q):
            bias = consts.tile([P, 4], f32, tag=f"b_{wtag}", name=f"b_{wtag}")
            w = consts.tile([P, 8, DG], bf16, tag=f"w_{wtag}", name=f"w_{wtag}")
            q.dma_start(bias[:], bdram.ap())
            q.dma_start(w[:], wdram.ap().rearrange("(c p) m -> p c m", p=P))
            return w, bias

        wk, bk = w_bias(wkT, bkd, "k", nc.sync)
        wq, bq = w_bias(wqT, bqd, "q", nc.gpsimd)
        xkh, xqh, xvh = alloc_x("xk_t"), alloc_x("xq_t"), alloc_x("xv_t")
        load_x_st(xkT, xkh, 0)
        load_x_st(xqT, xqh, 0)
        load_x_st(xqT, xqh, 1)
        wv, _ub = w_bias(wvT, bvd, "v0", nc.sync)
        bvrow = consts.tile([1, DG], bf16, tag="bvrow")
        nc.gpsimd.dma_start(bvrow[:], bvd.ap())
        bvb = consts.tile([P, DG], bf16, tag="bvb")
        nc.gpsimd.partition_broadcast(bvb[:], bvrow[:])
        load_x_st(xvT, xvh, 0)
        for st in range(1, 4):
            load_x_st(xkT, xkh, st)
            load_x_st(xvT, xvh, st)
            if st >= 2:
                load_x_st(xqT, xqh, st)
        wo_box = []

        def load_wo():
            wot = xin.tile([P, 8, S], bf16, tag="x", name="wo_t")
            nc.gpsimd.dma_start(
                wot[:, 0:4, 0:DIN],
                woT.ap().rearrange("(c p) m -> p c m", p=P))
            wo_box.append(wot)

        # ---- streamed work items ----
        def proj_group(w, bias, halves, dstT, c, st):
            pt = pq.tile([P, 512], f32, tag="qp", name=f"pj_{c}_{st}")
            for kc in range(8):
                nc.tensor.matmul(
                    pt[:],
                    w[:, kc, c * P:(c + 1) * P],
                    halves[:, kc, st * 512:(st + 1) * 512],
                    start=(kc == 0), stop=(kc == 7),
                )
            nc.vector.tensor_scalar_add(
                dstT[:, c, st * 512:(st + 1) * 512], pt[:],
                bias[:, c:c + 1])

        def v_group(sc):
            pt = pq.tile([P, DG], f32, tag="qp", name=f"pv_{sc}")
            for kc in range(8):
                nc.tensor.matmul(
                    pt[:],
                    xvh[:, kc, sc * P:(sc + 1) * P],
                    wv[:, kc, :],
                    start=(kc == 0), stop=(kc == 7),
                )
            nc.vector.tensor_tensor(vt[:, sc, :], pt[:], bvb[:], add_op)

        def o_group(st, nh, scalar_evac=False, use_alt=False):
            pool = psA if use_alt else pq
            po = pool.tile([P, 512], f32, tag="sA" if use_alt else "qp",
                           name=f"po_{st}_{nh}")
            wo = wo_box[0]
            for c in range(4):
                nc.tensor.matmul(
                    po[:],
                    OT[:, c, st * P:(st + 1) * P],
                    wo[:, c, nh * 512:(nh + 1) * 512],
                    start=(c == 0), stop=(c == 3))
            ob = osbp.tile([P, 512], bf16, tag="ob")
            if scalar_evac:
                nc.scalar.activation(ob[:], po[:], Copy)
            else:
                nc.vector.tensor_copy(ob[:], po[:])
            nc.sync.dma_start(
                outp.ap()[st * P:(st + 1) * P, nh * 512:(nh + 1) * 512],
                ob[:])

        # Work queue: (deadline_block, deadline_kc, ready_block, emit_fn).
        work = []
        for st in range(4):
            for c in range(4):
                if (c, st) != (0, 0):
                    dl = (0, 4 * st - 2) if c == 0 else (c - 1, 8 + 2 * st)
                    work.append((dl[0], dl[1], 0, lambda c=c, st=st:
                                 proj_group(wk, bk, xkh, KT, c, st)))
        for st in range(4):
            for c in range(4):
                if (c, st) in ((0, 0), (0, 1)):
                    continue
                b0 = 4 * (st // 2) + c
                work.append((max(0, b0 - 1), 6 + 2 * (st % 2), 0,
                             lambda c=c, st=st:
                             proj_group(wq, bq, xqh, QT, c, st)))
        for sc in range(16):
            work.append((0, max(0, sc - 1), 0, lambda sc=sc: v_group(sc)))
        work.append((3, 8, 3, lambda: load_wo()))
        for qt2 in range(2):
            for st in range(8 * qt2, 8 * qt2 + 8):
                for nh in range(2):
                    work.append((7, 99, 4 * qt2 + 5,
                                 lambda st=st, nh=nh, **kw:
                                 o_group(st, nh, **kw)))
        work.sort(key=lambda t: (t[0], t[1]))

        def pump(b, kc, budget):
            while work and (work[0][0], work[0][1]) <= (b, kc + 1):
                work.pop(0)[3]()
                budget -= 1
            while budget > 0:
                for i, (db, dk, rb, fn) in enumerate(work):
                    if rb <= b:
                        work.pop(i)[3]()
                        break
                else:
                    break
                budget -= 1
            return budget

        # head phase
        proj_group(wk, bk, xkh, KT, 0, 0)
        proj_group(wq, bq, xqh, QT, 0, 0)
        proj_group(wq, bq, xqh, QT, 0, 1)
        if not STREAM:
            keep = []
            for db, dk, rb, fn in work:
                if rb >= 4:
                    keep.append((99, 99, rb, fn))
                else:
                    fn()
            work.clear()
            work.extend(keep)

        # exp scratch (offload)
        def emit_exp(at_, st_, slot, par):
            off = (OFFLOAD_MOD and slot % OFFLOAD_MOD == 1
                   and par == (slot // OFFLOAD_MOD) % 2)
            if off:
                rp = attnp.tile([P, 1024], bf16, tag="at", name="rp_t")
                nc.vector._custom_dve(RP, out=rp[:], in0=st_[:],
                                      s0=LOG2E_8, s1=MAGIC, imm2=LN2_8)
                nc.vector._custom_dve(QS, out=at_[:], in0=rp[:],
                                      s0=0.0625, s1=0.5)
                for h2 in range(2):
                    hs = slice(h2 * 512, (h2 + 1) * 512)
                    pk = dvp.tile([P, 512], f32, tag="pk", name="pk_t")
                    nc.vector._custom_dve(PK, out=pk[:].bitcast(i32),
                                          in0=st_[:, hs],
                                          s0=C0B, s1=C1B, imm2=C2B)
                    nc.vector.tensor_tensor(at_[:, hs], at_[:, hs], pk[:],
                                            mult)
            else:
                nc.scalar.activation(at_[:], st_[:], Exp, scale=0.125)

        # ---- attention: 8 blocks of (q-half, head pair) ----
        LAG = 2
        deferred = []

        for b in range(8):
            qt2, p = b // 4, b % 4
            av0 = pav.tile([P, 512], f32, tag="av0", name=f"av0_{b}")
            av1 = pav.tile([P, 512], f32, tag="av1", name=f"av1_{b}")
            dn = pdn.tile([P, 512], f32, tag="dn", name=f"dn_{b}")
            nc.vector.memset(dn[:], 0.0)
            for kc in range(16):
                # alternate banks by kc parity: scores(kc+1) then wait on
                # exp(kc-1) -- long done -- instead of exp(kc), so the PE
                # never stalls on ScalarE and pairs co-issue.
                pA_, pB_ = (psA, psB) if kc % 2 == 0 else (psB, psA)
                stA = pA_.tile([P, 1024], f32, tag="sA" if kc % 2 == 0 else "sB")
                stB = pB_.tile([P, 1024], f32, tag="sB" if kc % 2 == 0 else "sA")
                kcs = slice(kc * P, (kc + 1) * P)
                for j in range(2):
                    qsl = slice(qt2 * 1024 + j * 512,
                                qt2 * 1024 + (j + 1) * 512)
                    nc.tensor.matmul(stB[:, j * 512:(j + 1) * 512],
                                     KT[64:128, p, kcs], QT[64:128, p, qsl],
                                     start=True, stop=True)
                    nc.tensor.matmul(stA[:, j * 512:(j + 1) * 512],
                                     KT[0:64, p, kcs], QT[0:64, p, qsl],
                                     start=True, stop=True)
                atA = attnp.tile([P, 1024], bf16, tag="at", name="atA_t")
                atB = attnp.tile([P, 1024], bf16, tag="at", name="atB_t")
                slot = b * 16 + kc
                emit_exp(atA, stA, slot, 0)
                emit_exp(atB, stB, slot, 1)

                def avs(av0=av0, av1=av1, dn=dn, atA=atA, atB=atB,
                        kc=kc, p=p):
                    vA = vt[:, kc, (2 * p) * HD:(2 * p + 1) * HD]
                    vB = vt[:, kc, (2 * p + 1) * HD:(2 * p + 2) * HD]
                    st_, sp_ = kc == 0, kc == 15
                    nc.tensor.matmul(av0[64:128, :], vB, atB[:, 0:512],
                                     start=st_, stop=sp_,
                                     skip_group_check=True)
                    nc.tensor.matmul(av0[0:64, :], vA, atA[:, 0:512],
                                     start=st_, stop=sp_,
                                     skip_group_check=True)
                    nc.tensor.matmul(av1[64:128, :], vB, atB[:, 512:1024],
                                     start=st_, stop=sp_,
                                     skip_group_check=True)
                    nc.tensor.matmul(av1[0:64, :], vA, atA[:, 512:1024],
                                     start=st_, stop=sp_,
                                     skip_group_check=True)
                    for r, at_, c0 in ((64, atB, 0), (96, atB, 512),
                                       (0, atA, 0), (32, atA, 512)):
                        nc.tensor.matmul(dn[r:r + 1, :], ones[:, 0:1],
                                         at_[:, c0:c0 + 512],
                                         start=st_, stop=sp_,
                                         tile_position=(0, r),
                                         skip_group_check=True)
                deferred.append(avs)
                if kc % 2 == 1:
                    while len(deferred) > LAG or (
                            deferred and getattr(deferred[0], "is_norm", 0)):
                        deferred.pop(0)()
                    pump(b, kc, 1)

            def norm(av0=av0, av1=av1, dn=dn, p=p, qt2=qt2, b=b):
                # denom rows -> sbuf@p0, approx-recip, bcast, multiply
                dncp = smallp.tile([P, 512], f32, tag="dncp", name="dncp")
                nc.vector.tensor_copy(dncp[:], dn[:])
                # rows: 0=(A,j0) 32=(A,j1) 64=(B,j0) 96=(B,j1)
                for i, r in enumerate((0, 32, 64, 96)):
                    # DMA the denom row to partition 0 (recip out and bcast
                    # src must sit at base partition 0), approx-recip, bcast
                    # into the [64,512] multiplier tile, then normalize.
                    par, j = i // 2, i % 2
                    di = smallp.tile([1, 512], f32, tag="di", name="di")
                    nc.sync.dma_start(di[:], dncp[r:r + 1, :])
                    bcs = smallp.tile([HD, 512], f32, tag="bcs", name="bcs")
                    nc.vector.reciprocal_approx_fast(out=bcs[0:1, :],
                                                     in_=di[:])
                    bc = smallp.tile([HD, 512], f32, tag="bcd", name="bcd")
                    nc.gpsimd.partition_broadcast(bc[:], bcs[0:1, :])
                    av = (av0, av1)[j]
                    qsl = slice(qt2 * 1024 + j * 512,
                                qt2 * 1024 + (j + 1) * 512)
                    nc.vector.tensor_tensor(
                        OT[par * 64:(par + 1) * 64, p, qsl],
                        av[par * 64:(par + 1) * 64, :], bc[:], mult)
            norm.is_norm = 1
            deferred.append(norm)

        for fn in deferred:
            fn()
        for i, (db, dk, rb, fn) in enumerate(work):
            try:
                fn(scalar_evac=(i % 2 == 0), use_alt=bool(i % 2))
            except TypeError:
                fn()
        work.clear()


def make_in_maps(q, k, v, Wq, bq, Wk, bk, Wv, bv, Wo, bo):
    bf = ml_dtypes.bfloat16
    in_maps = []
    for c in range(N_CORES):
        b_, g = c // 2, c % 2
        sl = slice(g * DG, (g + 1) * DG)
        in_maps.append({
            "xqT": np.ascontiguousarray(q[b_].T).astype(bf),
            "xkT": np.ascontiguousarray(k[b_].T).astype(bf),
            "xvT": np.ascontiguousarray(v[b_].T).astype(bf),
            "wqT": np.ascontiguousarray(Wq[sl].T).astype(bf),
            "wkT": np.ascontiguousarray(Wk[sl].T).astype(bf),
            "wvT": np.ascontiguousarray(Wv[sl].T).astype(bf),
            "woT": np.ascontiguousarray(Wo[:, sl].T).astype(bf),
            "bq": np.ascontiguousarray(
                bq[sl].astype(np.float32).reshape(4, P).T),
            "bk": np.ascontiguousarray(
                bk[sl].astype(np.float32).reshape(4, P).T),
            "bv": np.ascontiguousarray(
                bv[sl].astype(np.float32).reshape(1, DG)),
        })
    return in_maps


def assemble(results, bo):
    out = np.zeros((4, S, DIN), np.float32)
    for b_ in range(4):
        out[b_] = (results[2 * b_]["outp"].astype(np.float32)
                   + results[2 * b_ + 1]["outp"].astype(np.float32))
    out += np.asarray(bo, np.float32)[None, None, :]
    return out


def kernel(q, k, v, Wq, bq, Wk, bk, Wv, bv, Wo, bo):
    from concourse.bass_utils import run_bass_kernel_spmd

    if "nc" not in _CACHE:
        _CACHE["nc"] = build_bass()
    nc = _CACHE["nc"]
    in_maps = make_in_maps(q, k, v, Wq, bq, Wk, bk, Wv, bv, Wo, bo)
    res = run_bass_kernel_spmd(nc, in_maps, core_ids=list(range(N_CORES)))
    return assemble(res.results, bo)


# revision 8
# speedup vs baseline: 1.0822x; 1.0822x over previous
"""Multihead attention (B=4, S=2048, D=1024, H=16) on 8 Trainium2 NeuronCores.

Sharding: data-parallel over batch (4) x tensor-parallel over heads (2 groups
of 8 heads). Core c handles batch c//2, head-group c%2. Q/K/V projections are
column-parallel, attention fully local per head, out-projection row-parallel
producing a partial [S, D] output; two partials per batch are summed on host.

V2 design (vs the per-head baseline), all bf16 (fp8 anywhere in the
attention path fails the 2e-2 gate: attention outputs are averages of
zero-mean V, so element quantization noise passes straight through as
relative output error):
  - Score matmuls for a HEAD PAIR (2p, 2p+1) write one [128,1024] PSUM tile
    (cols 0:512 = head 2p, 512:1024 = head 2p+1) via two K=64 matmuls on PE
    row-tiles T0/T8 (64x128 mode) that can execute concurrently on HW.
  - One 1024-wide exp per kc on ScalarE (the roofline engine: 256 exps of
    (1024+352)/1.2 ns each).
  - attn.V matmuls are emitted LAG kc-slots late and popped in pairs at odd
    kc, so 64x128-mode scores and 128x128-mode AV/projection matmuls
    alternate every 2 kc (half the PE array mode switches of per-kc
    alternation) and block boundaries stay pipelined.
  - All projection / out-projection groups are streamed underneath the
    attention phase by a deadline-driven greedy scheduler; input x/w DMAs
    are staged s-tile-wise in first-use order; wo is loaded late into the
    recycled xk SBUF buffer.

Per-block PSUM (8 banks): "s" [128,1024]f32 x2 (4), "av" [65,512]f32 x2 (2),
"qp" [128,512]f32 x2 (2).

HW gotcha (cost 2 debug cycles): nc.vector.reciprocal output and
gpsimd.partition_broadcast source must sit at SBUF base partition 0 —
base-partition-64 slices pass CoreSim but corrupt on hardware.
"""

import sys

if "/opt/trn_rl_repo" not in sys.path:
    sys.path.insert(0, "/opt/trn_rl_repo")

import math

import numpy as np
import ml_dtypes

P = 128
S = 2048
DIN = 1024
DG = 512          # per-core projection width (8 heads * 64)
HD = 64
NH_LOCAL = 8      # heads per core
N_CORES = 8
VA = HD + 1       # per-head V_aug width (64 values + ones column)
LOG_SC = math.log(16.0)  # exp output scaled by 1/16 to fit fp8 e4m3 range

STREAM = True     # pump projections under attention
XW_FP8 = False     # x + q/k/v weights in fp8, projections via DoubleRow
OPROJ_FP8 = False  # OT + wo in fp8, out-projection via DoubleRow

_CACHE: dict = {}


def build_bass(repeat: int = 1):
    """Build the SPMD single-core program (same program on all 8 cores)."""
    from concourse import bacc, tile, mybir

    f32 = mybir.dt.float32
    bf16 = mybir.dt.bfloat16
    f8 = mybir.dt.float8e4
    xdt = f8 if XW_FP8 else bf16
    odt = f8 if OPROJ_FP8 else bf16

    nc = bacc.Bacc("TRN2", target_bir_lowering=False, debug=False,
                   num_devices=N_CORES)

    xqT = nc.dram_tensor("xqT", [DIN, S], xdt, kind="ExternalInput")
    xkT = nc.dram_tensor("xkT", [DIN, S], xdt, kind="ExternalInput")
    xvT = nc.dram_tensor("xvT", [DIN, S], xdt, kind="ExternalInput")
    wqT = nc.dram_tensor("wqT", [DIN, DG], xdt, kind="ExternalInput")
    wkT = nc.dram_tensor("wkT", [DIN, DG], xdt, kind="ExternalInput")
    wvT = nc.dram_tensor("wvT", [DIN, DG], xdt, kind="ExternalInput")
    woT = nc.dram_tensor("woT", [DG, DIN], odt, kind="ExternalInput")
    bqd = nc.dram_tensor("bq", [P, 4], f32, kind="ExternalInput")
    bkd = nc.dram_tensor("bk", [P, 4], f32, kind="ExternalInput")
    bvd = nc.dram_tensor("bv", [1, DG], f32, kind="ExternalInput")
    outp = nc.dram_tensor("outp", [S, DIN], f32, kind="ExternalOutput")

    with tile.TileContext(nc) as tc:
        for _ in range(repeat):
            _emit(nc, tc, xqT, xkT, xvT, wqT, wkT, wvT, woT, bqd, bkd, bvd,
                  outp)
    nc.compile()
    return nc


def _emit(nc, tc, xqT, xkT, xvT, wqT, wkT, wvT, woT, bqd, bkd, bvd, outp):
    from concourse import mybir

    f32 = mybir.dt.float32
    bf16 = mybir.dt.bfloat16
    f8 = mybir.dt.float8e4
    f8e5 = mybir.dt.float8e5
    xdt = f8 if XW_FP8 else bf16
    odt = f8 if OPROJ_FP8 else bf16
    Exp = mybir.ActivationFunctionType.Exp
    Copy = mybir.ActivationFunctionType.Copy
    mult = mybir.AluOpType.mult
    add_op = mybir.AluOpType.add
    DR = mybir.MatmulPerfMode.DoubleRow

    with (
        tc.tile_pool(name="consts", bufs=1) as consts,
        tc.tile_pool(name="xin", bufs=3) as xin,
        tc.tile_pool(name="qkv", bufs=1) as qkvp,
        tc.tile_pool(name="attn", bufs=6) as attnp,
        tc.tile_pool(name="small", bufs=1) as smallp,
        tc.tile_pool(name="osb", bufs=2) as osbp,
        tc.tile_pool(name="ps", bufs=2, space="PSUM") as psp,
        tc.tile_pool(name="pav", bufs=1, space="PSUM") as pav,
    ):
        QT = qkvp.tile([P, 4, S], bf16, tag="QT")
        KT = qkvp.tile([P, 4, S], bf16, tag="KT")
        # V_aug: [128 kpos, 16 kc, 8 heads * 65] (64 V dims + ones col)
        vaug = qkvp.tile([P, 16, NH_LOCAL * VA], bf16, tag="vaug")
        OT = qkvp.tile([P, 4, S], odt, tag="OT")

        # ones columns of V_aug (per head, both parities, all kc-pairs)
        v4 = vaug[:].rearrange("p a (h f) -> p (a h) f", f=VA)
        nc.vector.memset(v4[:, :, HD:HD + 1], 1.0)

        def alloc_x():
            return xin.tile([P, 8, S], xdt, tag="x", name="x_t")

        def load_x_st(xdram, xt_sb, st):
            # one s-tile (512 cols) of x^T, all 8 din-chunks, 2 DMA queues
            xt = xdram.ap().rearrange("(c p) m -> p c m", p=P)
            sl = slice(st * 512, (st + 1) * 512)
            nc.sync.dma_start(xt_sb[:, 0:4, sl], xt[:, 0:4, sl])
            nc.gpsimd.dma_start(xt_sb[:, 4:8, sl], xt[:, 4:8, sl])

        def w_bias(wdram, bdram, wtag, q):
            bias = consts.tile([P, 4], f32, tag=f"b_{wtag}")
            w = consts.tile([P, 8, DG], xdt, tag=f"w_{wtag}")
            q.dma_start(bias[:], bdram.ap())
            q.dma_start(w[:], wdram.ap().rearrange("(c p) m -> p c m", p=P))
            return w, bias

        # ---- input DMAs: weights, then x s-tile-interleaved (k,q,v) so the
        # first K/Q/V projection groups unblock as early as possible ----
        wk, bk = w_bias(wkT, bkd, "k", nc.sync)
        wq, bq = w_bias(wqT, bqd, "q", nc.gpsimd)
        xkh, xqh, xvh = alloc_x(), alloc_x(), alloc_x()
        load_x_st(xkT, xkh, 0)
        load_x_st(xqT, xqh, 0)
        load_x_st(xqT, xqh, 1)
        wv, _unused_bv = w_bias(wvT, bvd, "v0", nc.sync)
        bvrow = consts.tile([1, DG], bf16, tag="bvrow")
        nc.gpsimd.dma_start(bvrow[:], bvd.ap())
        bvb = consts.tile([P, DG], bf16, tag="bvb")
        nc.gpsimd.partition_broadcast(bvb[:], bvrow[:])
        bvb3 = bvb[:].rearrange("p (h f) -> p h f", f=HD)
        load_x_st(xvT, xvh, 0)
        for st in range(1, 4):
            load_x_st(xkT, xkh, st)
            load_x_st(xvT, xvh, st)
            if st >= 2:
                load_x_st(xqT, xqh, st)
        wo_box = []

        def load_wo():
            # recycles the xk buffer (xin pool, FIFO): emitted after the
            # last K-projection group has consumed xkh, well before the
            # first out-projection group reads it in block 4.
            wot = xin.tile([P, 8, S], xdt, tag="x", name="wo_t")
            nc.gpsimd.dma_start(
                wot[:, 0:4, 0:DIN],
                woT.ap().rearrange("(c p) m -> p c m", p=P))
            wo_box.append(wot)

        # ---- streamed work items (emitted under the attention phase) ----
        def proj_group(w, bias, halves, dstT, c, st):
            # dstT[dq, s] for dq chunk c, s-tile st (one PSUM group)
            pt = psp.tile([P, 512], f32, tag="qp", name=f"pj_{c}_{st}")
            if XW_FP8:
                for c2 in range(4):
                    half, loc = c2 // 2, (c2 % 2) * 2
                    nc.tensor.matmul(
                        pt[:],
                        w[:, 2 * c2:2 * c2 + 2, c * P:(c + 1) * P],
                        halves[:, 2 * c2:2 * c2 + 2,
                                     st * 512:(st + 1) * 512],
                        start=(c2 == 0), stop=(c2 == 3), perf_mode=DR,
                    )
            else:
                for kc in range(8):
                    nc.tensor.matmul(
                        pt[:],
                        w[:, kc, c * P:(c + 1) * P],
                        halves[:, kc, st * 512:(st + 1) * 512],
                        start=(kc == 0), stop=(kc == 7),
                    )
            nc.vector.tensor_scalar_add(
                dstT[:, c, st * 512:(st + 1) * 512], pt[:],
                bias[:, c:c + 1])

        def v_group(sc):
            # V projected directly in [kpos, dv] layout (x^T chunks
            # stationary); bias added via partition-broadcast row; fp8 out.
            pt = psp.tile([P, DG], f32, tag="qp", name=f"pv_{sc}")
            if XW_FP8:
                for c2 in range(4):
                    half, loc = c2 // 2, (c2 % 2) * 2
                    nc.tensor.matmul(
                        pt[:],
                        xvh[:, 2 * c2:2 * c2 + 2, sc * P:(sc + 1) * P],
                        wv[:, 2 * c2:2 * c2 + 2, :],
                        start=(c2 == 0), stop=(c2 == 3), perf_mode=DR,
                    )
            else:
                for kc in range(8):
                    nc.tensor.matmul(
                        pt[:],
                        xvh[:, kc, sc * P:(sc + 1) * P],
                        wv[:, kc, :],
                        start=(kc == 0), stop=(kc == 7),
                    )
            dst3 = vaug[:, sc].rearrange(
                "p (h f) -> p h f", f=VA)[:, :, 0:HD]
            src3 = pt[:].rearrange("p (h f) -> p h f", f=HD)
            nc.vector.tensor_tensor(dst3, src3, bvb3, add_op)

        def o_group(st, nh, scalar_evac=False, ptag="qp"):
            # partial[s, dout] = sum_dq OT[dq, s] * woT[dq, dout]
            po = psp.tile([P, 512], f32, tag=ptag, name=f"po_{st}_{nh}")
            wo = wo_box[0]
            for c in range(4):
                nc.tensor.matmul(
                    po[:],
                    OT[:, c, st * P:(st + 1) * P],
                    wo[:, c, nh * 512:(nh + 1) * 512],
                    start=(c == 0), stop=(c == 3))
            ob = osbp.tile([P, 512], f32, tag="ob")
            if scalar_evac:
                # tail phase: ScalarE is idle there, DVE/PE are not
                nc.scalar.activation(ob[:], po[:], Copy)
            else:
                nc.vector.tensor_copy(ob[:], po[:])
            nc.sync.dma_start(
                outp.ap()[st * P:(st + 1) * P, nh * 512:(nh + 1) * 512],
                ob[:])

        # Work queue: (deadline_block, deadline_kc, ready_block, emit_fn).
        # Forced emission once (block, kc+1) reaches the deadline; emitted
        # early (1 item/kc) when ready and the block has spare PE time.
        work = []
        for st in range(4):
            for c in range(4):
                if (c, st) != (0, 0):
                    # K chunk c s-tile st: first read by scores at
                    # (block 2c, kc 4*st). For c=0 emit just-in-time inside
                    # block 0 (the DMA stream is still delivering x there);
                    # otherwise spread across the preceding block.
                    dl = (0, 4 * st - 2) if c == 0 else                         (2 * c - 1, 4 + 2 * st)
                    work.append((dl[0], dl[1], 0, lambda c=c, st=st:
                                 proj_group(wk, bk, xkh, KT, c, st)))
        for st in range(4):
            for c in range(4):
                if (c, st) in ((0, 0), (0, 1)):
                    continue  # pre-emitted in the head phase
                # Q chunk c half st//2: first read by block
                # (qt2=st//2, h=2c) = 8*(st//2) + 2c.
                db = 8 * (st // 2) + 2 * c - 1
                work.append((db, 6 + 2 * (st % 2), 0, lambda c=c, st=st:
                             proj_group(wq, bq, xqh, QT, c, st)))
        for sc in range(16):
            # V s-chunk sc: first read by deferred AV(sc) at kc=sc+LAG.
            work.append((0, max(0, sc - 1), 0, lambda sc=sc: v_group(sc)))
        work.append((3, 6, 3, lambda: load_wo()))
        for qt2 in range(2):
            for st in range(8 * qt2, 8 * qt2 + 8):
                for nh in range(2):
                    # outproj s-tile st: every head finished q-half qt2 and
                    # (with LAG=4) the deferred normalizes are emitted one
                    # block into the next sweep.
                    work.append((15, 99, 8 * qt2 + 9,
                                 lambda st=st, nh=nh, **kw:
                                 o_group(st, nh, **kw)))
        work.sort(key=lambda t: (t[0], t[1]))

        def pump(b, kc, budget):
            # forced: everything whose deadline is within one kc of now
            while work and (work[0][0], work[0][1]) <= (b, kc + 1):
                work.pop(0)[3]()
                budget -= 1
            # opportunistic: ready items, up to remaining budget
            while budget > 0:
                for i, (db, dk, rb, fn) in enumerate(work):
                    if rb <= b:
                        work.pop(i)[3]()
                        break
                else:
                    break
                budget -= 1
            return budget

        # head phase: K(0,0) + Q(0,0..1) so the first scores can start
        proj_group(wk, bk, xkh, KT, 0, 0)
        proj_group(wq, bq, xqh, QT, 0, 0)
        proj_group(wq, bq, xqh, QT, 0, 1)
        if not STREAM:
            # bisection mode: emit everything up-front, nothing streamed
            keep = []
            for db, dk, rb, fn in work:
                if rb >= 4:  # outproj: after the blocks
                    keep.append((99, 99, rb, fn))
                else:
                    fn()
            work.clear()
            work.extend(keep)

        # ---- attention: 16 blocks of (qtile, head-pair), 512 q each ----
        # AV matmuls and the per-block normalize are emitted LAG kc-slots
        # late (crossing block boundaries) so the next block's score matmuls
        # and exps are already in the engine queues when a block ends.
        LAG = 5
        deferred = []  # FIFO of closures, popped LAG slots later

        for b in range(16):
            qt2, h = b // 8, b % 8
            cq, off = h // 2, (h % 2) * HD
            av = pav.tile([VA, 1024], f32, tag="av", name=f"av_{b}")
            for kc in range(16):
                st_ = psp.tile([P, 1024], f32, tag="s")
                for j in range(2):
                    nc.tensor.matmul(
                        st_[:, j * 512:(j + 1) * 512],
                        KT[off:off + HD, cq, kc * P:(kc + 1) * P],
                        QT[off:off + HD, cq,
                           qt2 * 1024 + j * 512:qt2 * 1024 + (j + 1) * 512],
                        start=True, stop=True)
                at = attnp.tile([P, 1024], bf16, tag="at")
                nc.scalar.activation(at[:], st_[:], Exp, scale=0.125)

                def av_mm(av=av, at=at, kc=kc, h=h):
                    for j in range(2):
                        nc.tensor.matmul(
                            av[:, j * 512:(j + 1) * 512],
                            vaug[:, kc, h * VA:(h + 1) * VA],
                            at[:, j * 512:(j + 1) * 512],
                            start=(kc == 0), stop=(kc == 15))
                deferred.append(av_mm)
                # pop deferred AVs in pairs at odd kc (and pump alongside)
                # so 64x128-mode scores and 128x128-mode AV/proj matmuls
                # alternate every 2 kc instead of every kc: half the PE
                # array mode switches. Normalizes pop eagerly so the
                # single-buffered av tile frees early in the next block.
                if kc % 2 == 1:
                    while len(deferred) > LAG or (
                            deferred and getattr(deferred[0], "is_norm", 0)):
                        deferred.pop(0)()
                    pump(b, kc, 1)
            def norm(av=av, h=h, qt2=qt2):
                # both half-chains interleaved so recip/broadcast/multiply
                # pipeline across DVE and GpSimd instead of serializing
                cq, off = h // 2, (h % 2) * HD
                rcs, bcs = [], []
                for j in range(2):
                    rc = smallp.tile([1, 512], f32, tag="rc",
                                     name=f"rc_{j}")
                    nc.vector.reciprocal(
                        rc[:], av[HD:HD + 1, j * 512:(j + 1) * 512])
                    rcs.append(rc)
                for j in range(2):
                    bc = smallp.tile([HD, 512], f32, tag="bc",
                                     name=f"bc_{j}")
                    nc.gpsimd.partition_broadcast(bc[:], rcs[j][0:1, :])
                    bcs.append(bc)
                for j in range(2):
                    nc.vector.tensor_tensor(
                        OT[off:off + HD, cq,
                           qt2 * 1024 + j * 512:qt2 * 1024 + (j + 1) * 512],
                        av[0:HD, j * 512:(j + 1) * 512], bcs[j][:], mult)
            norm.is_norm = 1
            deferred.append(norm)

        # drain deferred AVs/normalizes, then remaining work (last outproj)
        # with evacuations moved to the otherwise-idle ScalarE
        for fn in deferred:
            fn()
        for i, (db, dk, rb, fn) in enumerate(work):
            try:
                # the score-tile PSUM banks are dead in the drain: alternate
                # outproj accumulators onto them for 4-deep buffering
                fn(scalar_evac=(i % 2 == 0), ptag="s" if i % 2 else "qp")
            except TypeError:
                fn()
        work.clear()


def make_in_maps(q, k, v, Wq, bq, Wk, bk, Wv, bv, Wo, bo):
    bf = ml_dtypes.bfloat16
    f8 = ml_dtypes.float8_e4m3
    xdt = f8 if XW_FP8 else bf
    odt = f8 if OPROJ_FP8 else bf
    in_maps = []
    for c in range(N_CORES):
        b_, g = c // 2, c % 2
        sl = slice(g * DG, (g + 1) * DG)
        in_maps.append({
            "xqT": np.ascontiguousarray(q[b_].T).astype(xdt),
            "xkT": np.ascontiguousarray(k[b_].T).astype(xdt),
            "xvT": np.ascontiguousarray(v[b_].T).astype(xdt),
            "wqT": np.ascontiguousarray(Wq[sl].T).astype(xdt),
            "wkT": np.ascontiguousarray(Wk[sl].T).astype(xdt),
            "wvT": np.ascontiguousarray(Wv[sl].T).astype(xdt),
            "woT": np.ascontiguousarray(Wo[:, sl].T).astype(odt),
            "bq": np.ascontiguousarray(
                bq[sl].astype(np.float32).reshape(4, P).T),
            "bk": np.ascontiguousarray(
                bk[sl].astype(np.float32).reshape(4, P).T),
            "bv": np.ascontiguousarray(
                bv[sl].astype(np.float32).reshape(1, DG)),
        })
    return in_maps


def assemble(results, bo):
    out = np.zeros((4, S, DIN), np.float32)
    for b_ in range(4):
        out[b_] = results[2 * b_]["outp"] + results[2 * b_ + 1]["outp"]
    out += np.asarray(bo, np.float32)[None, None, :]
    return out


def kernel(q, k, v, Wq, bq, Wk, bk, Wv, bv, Wo, bo):
    from concourse.bass_utils import run_bass_kernel_spmd

    if "nc" not in _CACHE:
        _CACHE["nc"] = build_bass()
    nc = _CACHE["nc"]
    in_maps = make_in_maps(q, k, v, Wq, bq, Wk, bk, Wv, bv, Wo, bo)
    res = run_bass_kernel_spmd(nc, in_maps, core_ids=list(range(N_CORES)))
    return assemble(res.results, bo)

